# revision 75
# baseline (speedup 1.0000x reference)
"""Trainium2 Bass kernel for nn_MixerGroupedTiedAttention.

Sharding: 8 cores = (batch B=2) x (kv-group G=4). Each core handles one
batch element and one group of 4 q-heads + their shared kv-head:
  - qkv / gate projections: tensor-parallel column slices of W_qkv / W_g
  - k_rope (head-tied) replicated (folded into the per-core W slab)
  - sliding-window attention (W=1024) computed block-sparse over 128x128
    token tiles.

Design notes:
  - x is transposed to d-major on the HOST and shipped per token-tile,
    so the PE transposes + evac copies of a device-side transpose
    pipeline vanish.
  - Projections run as fp8e4m3 DoubleRow matmuls (0.5 PE cycles/row,
    2x the 16-bit rate) over 256-deep chunk pairs. Precision comes
    from a hi+lo split of BOTH operands (lo = fp8 of the residual;
    3 cross terms accumulate in PSUM, the ~1e-3 lo*lo term is
    dropped). Weights are pre-scaled by 64 so the lo plane clears
    e4m3's subnormal floor; rmsnorm makes q/kv scale-invariant and
    krope/gate divide the 64 back out in fused scalar ops. Attention
    stays fp16/bf16 (q/k fp16, probs/V bf16). rel err ~8.7e-3 vs the
    2e-2 gate.
  - Single fused pass per 128-token tile t: projections (tokens on PSUM
    partitions) -> rmsnorm/rope/scales -> transposes -> attention ROW
    t-1 at lag 1: per 128x128 key block one score matmul computes ALL
    4 heads (the 4-head q strip is the moving operand), exp'd into a
    per-block [j, (h,i)] bf16 strip; y accumulates per head over the 9
    blocks with a ones column fused into v giving the softmax
    denominator for free. The rmsnorm/rope chain + transposes run
    under tc.high_priority() — they are the cross-block critical path
    (they free the pq PSUM bank and gate the next row's scores).
  - The last row borrows the four then-dead projection PSUM banks to
    accumulate y key-block-major, so after the final exp strip only 4
    matmuls remain before the normalize/store.
  - DMA: HWDGE costs a serialized ~625ns per DMA, so transfers are
    batched >=1.5KB/partition; cost-model bandwidth is ~360GB/s shared.
"""

import numpy as np
import ml_dtypes

D_MODEL = 2048
N_HEADS = 16
N_KV = 4
D_HEAD = 128
D1 = 64
D2 = 64
WSIZE = 1024
EPS = 1e-6
ROPE_BASE = 10000.0
B = 2
T = 2048
NCORES = 8
HPC = 4  # q heads per core
NT = T // 128  # 16 token tiles
NWB = WSIZE // 128 + 1  # 9 key tiles per query tile
WCOLS = 1280  # q(512) | kv(128) | krope(64) | pad(64) | gate(512)

_BF16 = ml_dtypes.bfloat16
_FP8 = ml_dtypes.float8_e4m3  # matches mybir dt.float8e4
_built = {}


def _build_nc():
    """Build the single-core SPMD Bass program (same program all 8 cores)."""
    if "nc" in _built:
        return _built["nc"]
    import concourse.bacc as bacc
    import concourse.tile as tile
    from concourse import mybir

    # All ACT functions this kernel uses (Copy/Square/Ln/Exp) live in the
    # "natural_log_exp_and_others" table set. The table-load pass greedily
    # picks the first set containing each function, which alternates table
    # loads (~2.7us each) between sets; restrict every other set's
    # advertised membership so exactly one table set is ever loaded.
    if not getattr(bacc, "_act_tables_pinned", False):
        _orig_gat = bacc.get_activation_tables
        _mine = {
            mybir.ActivationFunctionType.Copy,
            mybir.ActivationFunctionType.Identity,
            mybir.ActivationFunctionType.Square,
            mybir.ActivationFunctionType.Ln,
            mybir.ActivationFunctionType.Exp,
        }

        def _pinned_gat(arch):
            tabs = _orig_gat(arch)
            return {
                name: (funcs if name == "natural_log_exp_and_others"
                       else funcs - _mine)
                for name, funcs in tabs.items()
            }

        bacc.get_activation_tables = _pinned_gat
        bacc._act_tables_pinned = True

    f32 = mybir.dt.float32
    f32r = mybir.dt.float32r
    f16 = mybir.dt.float16
    bf16 = mybir.dt.bfloat16
    AF = mybir.ActivationFunctionType
    OP = mybir.AluOpType

    nc = bacc.Bacc("TRN2", target_bir_lowering=False, debug=False)

    def din(name, shape, dt):
        return nc.dram_tensor(name, shape, dt, kind="ExternalInput").ap()

    fp8 = mybir.dt.float8e4
    DR = mybir.MatmulPerfMode.DoubleRow

    # x and W ship as fp8e4m3 hi+lo pairs (hi = quantized value, lo =
    # quantized residual); projections run as DoubleRow matmuls over
    # 256-deep chunk pairs with 3 cross terms (hh, hl, lh — the lo*lo
    # term is ~1e-3 relative and dropped), 1.33x faster than 16-bit.
    # x layout per tile: [128, hi(2048) | lo(2048)] d-major; weights
    # pair-major [128, pair, hi/lo, 2, cols] so a DoubleRow rhs slice
    # is contiguous. trig packs cos | sin | crow | brk; masks mdiag|mfar.
    xt = din("xt", [NT, 128, 2 * D_MODEL], fp8)
    wqkv = din("wqkv", [128, 8, 2, 2, 768], fp8)  # q|kv|krope|g0:64
    wg = din("wg", [128, 8, 2, 2, 448], fp8)      # gate cols 64:512
    trig = din("trig", [128, 1152], f32)
    masks = din("masks", [128, 1024], bf16)    # tiled x4 for fused heads
    ident = din("ident", [128, 128], f16)
    out = nc.dram_tensor("out", [T, 512], f32, kind="ExternalOutput").ap()

    with tile.TileContext(nc) as tc:
        with tc.tile_pool(name="persist", bufs=1) as pp:
            wq_sb = pp.tile([128, 8, 2, 2, 768], fp8, name="wq")
            wg_sb = pp.tile([128, 8, 2, 2, 448], fp8, name="wg")
            trig_sb = pp.tile([128, 1152], f32)
            mask_sb = pp.tile([128, 1024], bf16)
            ident_sb = pp.tile([128, 128], f16)
            eps_sb = pp.tile([128, 1], f32)
            kT_sb = pp.tile([128, T], f16)
            vaug_sb = pp.tile([128, NT, 132], bf16)
            nc.vector.memset(eps_sb[:], EPS)
            # ones column of v_aug (softmax-denominator accumulator)
            nc.vector.memset(vaug_sb[:, :, 128], 1.0)

            def cosv(t):
                return trig_sb[:, t * 32:(t + 1) * 32]

            def sinv(t):
                return trig_sb[:, 512 + t * 32:512 + (t + 1) * 32]

            def crowv(t):
                return trig_sb[:, 1024 + t * 4:1024 + (t + 1) * 4]

            brk_sb = trig_sb[:, 1088:1152]
            mdiag_sb = mask_sb[:, 0:512]
            mfar_sb = mask_sb[:, 512:1024]

            with tc.tile_pool(name="xt", bufs=8) as xtp, \
                 tc.tile_pool(name="qt", bufs=3) as qtp, \
                 tc.tile_pool(name="gs", bufs=4) as gsp, \
                 tc.tile_pool(name="u", bufs=19) as up, \
                 tc.tile_pool(name="aw", bufs=3) as awp, \
                 tc.tile_pool(name="qk", bufs=3) as qkp, \
                 tc.tile_pool(name="bw", bufs=3) as bwp, \
                 tc.tile_pool(name="stg", bufs=2) as stp, \
                 tc.tile_pool(name="ppq", bufs=2, space="PSUM") as ppq, \
                 tc.tile_pool(name="ppkv", bufs=1, space="PSUM") as ppkv, \
                 tc.tile_pool(name="ppg", bufs=1, space="PSUM") as ppg, \
                 tc.tile_pool(name="psS", bufs=4, space="PSUM") as psS:

                # ---- DMA schedule. HWDGE costs a serialized ~625ns per
                # DMA regardless of size, so batch: weight chunks stream in
                # pairs/quads, x tiles 0-2 in 1024-col halves, small
                # tensors packed (trig, masks). Cold start interleaves the
                # qkv weight stream with x tiles 0-2 just in time for the
                # trailing cold matmuls. ----
                xpre = {}
                TERMS = ((0, 0), (0, 1), (1, 0))  # (x hi/lo, w hi/lo)

                def fetch_x(t, pieces=1):
                    xs = xtp.tile([128, 2 * D_MODEL], fp8, name=f"x_{t}",
                                  tag="x")
                    xpre[t] = xs
                    for c in range(pieces):
                        w = 2 * D_MODEL // pieces
                        nc.sync.dma_start(xs[:, c * w:(c + 1) * w],
                                          xt[t][:, c * w:(c + 1) * w])

                for t in range(3):
                    fetch_x(t, pieces=0)  # allocate in consumption order
                xcold = [xpre[t] for t in range(3)]

                def xpiece(t, c):
                    # piece 0 = hi plane, piece 1 = lo plane (2KB each)
                    nc.sync.dma_start(xcold[t][:, c * 2048:(c + 1) * 2048],
                                      xt[t][:, c * 2048:(c + 1) * 2048])

                def xdr(x_slab, xi, P):
                    """DoubleRow stationary: chunk pair P of x hi/lo."""
                    base = xi * D_MODEL + P * 256
                    return x_slab[:, base:base + 256].rearrange(
                        "p (i c) -> p i c", i=2)

                def wqs(p0, p1):
                    nc.sync.dma_start(wq_sb[:, p0:p1], wqkv[:, p0:p1])

                wqs(0, 1)
                xpiece(0, 0)
                wqs(1, 2)
                xpiece(0, 1)
                wqs(2, 3)
                xpiece(1, 0)
                wqs(3, 4)
                xpiece(1, 1)
                wqs(4, 5)
                xpiece(2, 0)
                wqs(5, 6)
                xpiece(2, 1)
                wqs(6, 7)
                wqs(7, 8)
                nc.sync.dma_start(wg_sb[:, 0:2], wg[:, 0:2])
                nc.sync.dma_start(ident_sb[:], ident[:])
                nc.sync.dma_start(trig_sb[:], trig[:])
                nc.sync.dma_start(wg_sb[:, 2:4], wg[:, 2:4])
                fetch_x(3)
                nc.sync.dma_start(mask_sb[:], masks[:])
                nc.sync.dma_start(wg_sb[:, 4:6], wg[:, 4:6])
                fetch_x(4)
                nc.sync.dma_start(wg_sb[:, 6:8], wg[:, 6:8])
                fetch_x(5)
                fetch_x(6)
                for t in range(7, NT):
                    fetch_x(t)

                qf_t = {}
                kpre_t = {}
                gs_t = {}

                def dr_group(dst, x_slab, w_sb, c0, c1):
                    """One 256-deep-pair DoubleRow accumulation group."""
                    n = 0
                    for P in range(8):
                        for xi, wi in TERMS:
                            nc.tensor.matmul(
                                dst, xdr(x_slab, xi, P),
                                w_sb[:, P, wi, :, c0:c1],
                                start=(n == 0), stop=(n == 23),
                                perf_mode=DR)
                            n += 1

                def emit_proj_qkv(t):
                    """q/kv projection matmuls for tile t (PE only)."""
                    x_slab = xpre[t]
                    pq = ppq.tile([128, 512], f32, name=f"pq_{t}", tag="pq")
                    pkv = ppkv.tile([128, 256], f32, name=f"pk_{t}",
                                    tag="pk")
                    dr_group(pq[:, 0:256], x_slab, wq_sb, 0, 256)
                    dr_group(pq[:, 256:512], x_slab, wq_sb, 256, 512)
                    dr_group(pkv[:], x_slab, wq_sb, 512, 768)
                    return pq, pkv

                def emit_proj_gate(t):
                    """gate projection matmuls for tile t (PE only).
                    Gate cols 0:64 ride in the kvp region (former pad);
                    this computes cols 64:512."""
                    x_slab = xpre.pop(t)
                    pg = ppg.tile([128, 448], f32, name=f"pg_{t}", tag="pg")
                    dr_group(pg[:, 0:256], x_slab, wg_sb, 0, 256)
                    dr_group(pg[:, 256:448], x_slab, wg_sb, 256, 448)
                    return pg

                def emit_chain_qkv(t, pq, pkv):
                    """rmsnorm/rope/scales for tile t (no PE ops).

                    DVE order puts the rope muls (which depend only on the
                    just-landed PSUM + trig) before anything that needs the
                    ACT rmsnorm chain, so DVE and ACT start in parallel the
                    moment the projection completes."""
                    cos_t = cosv(t)
                    sin_t = sinv(t)
                    cosb = cos_t[:, None, :].broadcast_to([128, 4, 32])
                    sinb = sin_t[:, None, :].broadcast_to([128, 4, 32])
                    qh = pq[:].rearrange("p (h d) -> p h d", h=4)
                    t1 = awp.tile([128, 4, 32], f32, name=f"t1_{t}", tag="t1")
                    t2 = awp.tile([128, 4, 32], f32, name=f"t2_{t}", tag="t2")
                    t3 = awp.tile([128, 4, 32], f32, name=f"t3_{t}", tag="t3")
                    t4 = awp.tile([128, 4, 32], f32, name=f"t4_{t}", tag="t4")
                    rp = awp.tile([128, 4, 64], f32, name=f"rp_{t}", tag="rp")
                    nc.vector.tensor_mul(t1[:], qh[:, :, 64:96], cosb)
                    nc.vector.tensor_mul(t2[:], qh[:, :, 96:128], sinb)
                    nc.gpsimd.tensor_add(rp[:, :, 0:32], t1[:], t2[:])
                    nc.vector.tensor_mul(t3[:], qh[:, :, 96:128], cosb)
                    nc.vector.tensor_mul(t4[:], qh[:, :, 64:96], sinb)
                    nc.gpsimd.tensor_sub(rp[:, :, 32:64], t3[:], t4[:])
                    # k_rope: bias, rope (no norm)
                    krf = awp.tile([128, 64], f32, name=f"krf_{t}", tag="krf")
                    nc.vector.scalar_tensor_tensor(
                        krf[:], pkv[:, 128:192], 1.0 / 64.0, brk_sb,
                        OP.mult, OP.add)
                    k1 = awp.tile([128, 32], f32, name=f"k1_{t}", tag="k1")
                    k2 = awp.tile([128, 32], f32, name=f"k2_{t}", tag="k2")
                    k3 = awp.tile([128, 32], f32, name=f"k3_{t}", tag="k3")
                    k4 = awp.tile([128, 32], f32, name=f"k4_{t}", tag="k4")
                    kpre = qkp.tile([128, 128], f16, name=f"kp_{t}",
                                    tag="kp")
                    nc.vector.tensor_mul(k1[:], krf[:, 0:32], cos_t)
                    nc.vector.tensor_mul(k2[:], krf[:, 32:64], sin_t)
                    nc.gpsimd.tensor_add(kpre[:, 64:96], k1[:], k2[:])
                    nc.vector.tensor_mul(k3[:], krf[:, 32:64], cos_t)
                    nc.vector.tensor_mul(k4[:], krf[:, 0:32], sin_t)
                    nc.gpsimd.tensor_sub(kpre[:, 96:128], k3[:], k4[:])

                    # rmsnorm scales: sumsq over each 128-wide head chunk
                    # (chunks 0-3 = q heads, 4 = kv head); r = rsqrt(mean+
                    # eps) via exp(-0.5*ln(.)) — keeps every ACT function in
                    # the ln/exp table set
                    ss = awp.tile([128, 5], f32, name=f"ss_{t}", tag="ss")
                    sq = awp.tile([128, 128], f32, name=f"sq_{t}", tag="sq")
                    for hc in range(4):
                        nc.scalar.activation(
                            sq[:], pq[:, hc * 128:(hc + 1) * 128],
                            AF.Square, accum_out=ss[:, hc:hc + 1])
                    nc.scalar.activation(sq[:], pkv[:, 0:128], AF.Square,
                                         accum_out=ss[:, 4:5])
                    lnm = awp.tile([128, 5], f32, name=f"lnm_{t}", tag="lnm")
                    nc.scalar.activation(lnm[:], ss[:], AF.Ln,
                                         scale=1.0 / 128.0, bias=eps_sb[:])
                    r = awp.tile([128, 5], f32, name=f"r_{t}", tag="r")
                    nc.scalar.activation(r[:], lnm[:], AF.Exp, scale=-0.5)
                    rc = awp.tile([128, HPC], f32, name=f"rc_{t}", tag="rc")
                    nc.vector.tensor_mul(rc[:], r[:, 0:4], crowv(t))
                    qf = qkp.tile([128, 4, 128], f16, name=f"qf_{t}",
                                  tag="qf")
                    rcb = rc[:, :, None].broadcast_to([128, 4, 64])
                    nc.vector.tensor_mul(qf[:, :, 0:64], qh[:, :, 0:64], rcb)
                    nc.vector.tensor_mul(qf[:, :, 64:128], rp[:], rcb)
                    qf_t[t] = qf

                    # kv head -> v (token-major) and k tied half
                    nc.vector.tensor_scalar(
                        vaug_sb[:, t, 0:128], pkv[:, 0:128],
                        r[:, 4:5], None, OP.mult)
                    nc.vector.tensor_scalar(
                        kpre[:, 0:64], pkv[:, 0:64],
                        r[:, 4:5], None, OP.mult)
                    kpre_t[t] = kpre

                qT_t = {}

                def emit_trans(t):
                    """PE transposes of qf/kpre for tile t; evac on DVE
                    (qT) and Pool (kT slab) keeps ACT free for exps."""
                    qf = qf_t.pop(t)
                    kpre = kpre_t.pop(t)
                    tq = psS.tile([128, 640], f16, name=f"tq_{t}", tag="s")
                    for h in range(HPC):
                        nc.tensor.transpose(
                            tq[:, h * 128:(h + 1) * 128], qf[:, h, :],
                            ident_sb[:])
                    nc.tensor.transpose(tq[:, 512:640], kpre[:], ident_sb[:])
                    qT = qtp.tile([128, HPC, 128], f16, name=f"qT_{t}",
                                  tag="qT")
                    # kT first: the diag score block waits on it, while
                    # qT has the whole y(t-1) stretch of slack. DVE (not
                    # Pool): GPSIMD cannot read PSUM on real HW.
                    nc.vector.tensor_copy(kT_sb[:, t * 128:(t + 1) * 128],
                                          tq[:, 512:640])
                    nc.vector.tensor_copy(
                        qT[:], tq[:, 0:512].rearrange("p (h t) -> p h t",
                                                      h=4))
                    qT_t[t] = qT

                def emit_chain_gate(t, pkv, pg):
                    # gate: silu = g / (1 + exp(-g)) — exp keeps the single
                    # ACT table set; +1 on Pool; reciprocal+mul on DVE.
                    # g cols 0:64 come from the kvp PSUM (former pad cols).
                    gsg = awp.tile([128, 512], f32, name=f"gsg_{t}",
                                   tag="gsg")
                    nc.scalar.activation(gsg[:, 0:64], pkv[:, 192:256],
                                         AF.Exp, scale=-1.0 / 64.0)
                    nc.scalar.activation(gsg[:, 64:512], pg[:], AF.Exp,
                                         scale=-1.0 / 64.0)
                    gw = awp.tile([128, 512], f32, name=f"gw_{t}", tag="gw")
                    nc.gpsimd.tensor_scalar_add(gw[:], gsg[:], 1.0)
                    gwi = awp.tile([128, 512], f32, name=f"gwi_{t}",
                                   tag="gwi")
                    nc.vector.reciprocal(gwi[:], gw[:])
                    g16 = gsp.tile([128, 512], f16, name=f"gs_{t}", tag="gs")
                    nc.vector.scalar_tensor_tensor(
                        g16[:, 0:64], pkv[:, 192:256], 1.0 / 64.0,
                        gwi[:, 0:64], OP.mult, OP.mult)
                    nc.vector.scalar_tensor_tensor(
                        g16[:, 64:512], pg[:], 1.0 / 64.0,
                        gwi[:, 64:512], OP.mult, OP.mult)
                    gs_t[t] = g16

                ublks_t = {}

                def emit_b1s(r):
                    """Attention row r: scores + exp strips. Natural block
                    order — the diag block (needing the freshest kT slab)
                    goes last, giving the Pool kT evac time to land. The
                    mask multiplies are deferred to emit_b2 (on Pool) so
                    the DVE queue never stalls waiting for the last exp."""
                    qT = qT_t.pop(r)
                    j0 = max(0, r - (NWB - 1))
                    nblk = r - j0 + 1
                    # scores: per key block, ONE matmul for all 4 heads
                    # (moving = the 4-head q strip); exp to a [j,(h,i)] strip
                    ublks = [None] * nblk
                    for wi in range(nblk):
                        tj = j0 + wi
                        s_ps = psS.tile([128, 512], f32,
                                        name=f"s_{r}_{wi}", tag="s")
                        nc.tensor.matmul(
                            s_ps[:], kT_sb[:, tj * 128:(tj + 1) * 128],
                            qT[:], start=True, stop=True)
                        u_t = up.tile([128, 512], bf16, name=f"u_{r}_{wi}",
                                      tag="u")
                        nc.scalar.activation(u_t[:], s_ps[:], AF.Exp)
                        ublks[wi] = u_t
                    # masks trail all exps: far block first (its exp landed
                    # long ago), diag last
                    if nblk == NWB:
                        nc.vector.tensor_mul(ublks[0][:], ublks[0][:],
                                             mfar_sb)
                    nc.vector.tensor_mul(ublks[nblk - 1][:],
                                         ublks[nblk - 1][:], mdiag_sb)
                    ublks_t[r] = ublks

                def emit_b2(r, ypools=None):
                    """Attention row r: y accumulation, normalize, gate, out.

                    Default: head-major accumulation through the shared psS
                    rotation. For the last row, `ypools` borrows the four
                    then-dead projection banks so accumulation can run
                    key-block-major (4 concurrent groups, one per bank) —
                    after the final exp strip only 4 matmuls remain."""
                    j0 = max(0, r - (NWB - 1))
                    nblk = r - j0 + 1
                    ublks = ublks_t.pop(r)
                    gsr = gs_t.pop(r)
                    stage = stp.tile([128, 512], f32, name=f"o_{r}", tag="o")
                    if ypools is None:
                        yps = []
                        for h in range(HPC):
                            y_ps = psS.tile([128, 132], f32,
                                            name=f"y_{r}_{h}", tag="s")
                            yps.append(y_ps)
                            for wi in range(nblk):
                                nc.tensor.matmul(
                                    y_ps[:, 0:129],
                                    ublks[wi][:, h * 128:(h + 1) * 128],
                                    vaug_sb[:, j0 + wi, 0:129],
                                    start=(wi == 0), stop=(wi == nblk - 1))
                    else:
                        yps = [pool.tile([128, 132], f32, name=f"y_{r}_{h}",
                                         tag=tag)
                               for h, (pool, tag) in enumerate(ypools)]
                        for wi in range(nblk):
                            for h in range(HPC):
                                nc.tensor.matmul(
                                    yps[h][:, 0:129],
                                    ublks[wi][:, h * 128:(h + 1) * 128],
                                    vaug_sb[:, j0 + wi, 0:129],
                                    start=(wi == 0), stop=(wi == nblk - 1))
                    for h in range(HPC):
                        linv = bwp.tile([128, 1], f32, name=f"li_{r}_{h}",
                                        tag="li")
                        nc.vector.reciprocal(linv[:], yps[h][:, 128:129])
                        nc.vector.scalar_tensor_tensor(
                            stage[:, h * 128:(h + 1) * 128],
                            yps[h][:, 0:128], linv[:],
                            gsr[:, h * 128:(h + 1) * 128],
                            OP.mult, OP.mult)
                        if h == 1:
                            nc.sync.dma_start(
                                out[r * 128:(r + 1) * 128, 0:256],
                                stage[:, 0:256])
                    nc.sync.dma_start(out[r * 128:(r + 1) * 128, 256:512],
                                      stage[:, 256:512])

                # ---- cold start: tiles 0-2 q/kv chunk-major with
                # trailing offsets (PE tracks the weight stream); their
                # q/kv PSUM partly borrowed from the idle psS pool ----
                cold_q = [ppq.tile([128, 512], f32, name="cq_0", tag="pq"),
                          ppq.tile([128, 512], f32, name="cq_1", tag="pq"),
                          psS.tile([128, 512], f32, name="cq_2", tag="s")]
                cold_kv = [ppkv.tile([128, 256], f32, name="ck_0", tag="pk"),
                           psS.tile([128, 256], f32, name="ck_1", tag="s"),
                           psS.tile([128, 256], f32, name="ck_2", tag="s")]
                # interleave only the kv + q-lo-half groups while tracking
                # the weight stream (they accumulate in different PSUM
                # banks; the q halves share one bank so their groups must
                # be sequential), then burst the q-hi-half groups
                for i in range(12):
                    for t, trail in ((0, 0), (1, 2), (2, 4)):
                        P = i - trail
                        if not (0 <= P < 8):
                            continue
                        for dst, c0, c1 in (
                                (cold_kv[t][:], 512, 768),
                                (cold_q[t][:, 0:256], 0, 256)):
                            for xi, wi in TERMS:
                                nc.tensor.matmul(
                                    dst, xdr(xpre[t], xi, P),
                                    wq_sb[:, P, wi, :, c0:c1],
                                    start=(P == 0 and (xi, wi) == TERMS[0]),
                                    stop=(P == 7 and (xi, wi) == TERMS[-1]),
                                    perf_mode=DR)
                for t in range(3):
                    dr_group(cold_q[t][:, 256:512], xpre[t], wq_sb, 256, 512)
                # cold epilogue: chains + transposes for tiles 0-2 ordered
                # so the borrowed psS slots free in pool-rotation order,
                # then attention rows start. Steady state runs at lag 1 —
                # row t-1's scores/exps/y hide under tile t's projections —
                # so only row 15's attention remains after the last proj.
                emit_chain_qkv(0, cold_q[0], cold_kv[0])
                emit_trans(0)
                emit_chain_qkv(1, cold_q[1], cold_kv[1])
                pg0 = emit_proj_gate(0)
                emit_chain_gate(0, cold_kv[0][:], pg0)
                emit_chain_qkv(2, cold_q[2], cold_kv[2])
                emit_trans(1)
                pg1 = emit_proj_gate(1)
                emit_chain_gate(1, cold_kv[1][:], pg1)
                emit_trans(2)
                emit_b1s(0)
                pg2 = emit_proj_gate(2)
                emit_chain_gate(2, cold_kv[2][:], pg2)
                emit_b2(0)
                emit_b1s(1)
                emit_b2(1)
                for t in range(3, NT - 1):
                    emit_b1s(t - 1)
                    pq, pkv = emit_proj_qkv(t)
                    pg = emit_proj_gate(t)
                    emit_b2(t - 1)
                    # the rmsnorm/rope chain + transposes are the cross-
                    # block critical path (they free the pq bank and gate
                    # the next row's scores); high priority lets them
                    # preempt the exp strips in the ready heaps
                    with tc.high_priority():
                        emit_chain_qkv(t, pq, pkv)
                        emit_trans(t)
                    emit_chain_gate(t, pkv, pg)
                # t = 15: row-15 scores jump ahead of row 14's y and the
                # gate silu jumps ahead of the row-15 exps on ACT, so the
                # tail (row 15's exps + y + normalize) starts as early as
                # possible after the last projection
                emit_b1s(NT - 2)
                pq, pkv = emit_proj_qkv(NT - 1)
                pg = emit_proj_gate(NT - 1)
                with tc.high_priority():
                    emit_chain_qkv(NT - 1, pq, pkv)
                    emit_trans(NT - 1)
                emit_chain_gate(NT - 1, pkv, pg)
                emit_b1s(NT - 1)
                emit_b2(NT - 2)
                emit_b2(NT - 1, ypools=[(ppq, "pq"), (ppq, "pq"),
                                        (ppkv, "pk"), (ppg, "pg")])

    nc.compile()
    _built["nc"] = nc
    return nc


def _host_inputs(hidden_states, W_qkv, W_rk, b_rk, softmax_scaler, W_g):
    """Per-core input dicts (host-side sharding / layout / dtype prep)."""
    inv_freq = 1.0 / (ROPE_BASE ** (np.arange(0, D2, 2, dtype=np.float32) / D2))
    tpos = np.arange(T, dtype=np.float32)
    freqs = tpos[:, None] * inv_freq[None, :]
    cost = np.cos(freqs).astype(np.float32)
    sint = np.sin(freqs).astype(np.float32)
    logpos = np.log(np.minimum(tpos + 1.0, float(WSIZE))).astype(np.float32)
    scale = logpos / np.float32(np.sqrt(D_HEAD))

    ii = np.arange(128)
    mdiag = np.tile((ii[:, None] <= ii[None, :]).astype(_BF16), (1, 4))
    mfar = np.tile((ii[:, None] >= ii[None, :]).astype(_BF16), (1, 4))
    ident = np.eye(128, dtype=np.float16)
    brk_t = np.broadcast_to(
        np.asarray(b_rk, np.float32)[None, :], (128, 64)).copy()

    xf = np.asarray(hidden_states, np.float32)
    wqkv_f = np.asarray(W_qkv, np.float32)
    wrk_f = np.asarray(W_rk, np.float32)
    wg_f = np.asarray(W_g, np.float32)
    scaler = np.asarray(softmax_scaler, np.float32)
    zpad = np.zeros((D_MODEL, 64), np.float32)

    def hilo(a):
        hi = a.astype(_FP8)
        lo = (a - hi.astype(np.float32)).astype(_FP8)
        return hi, lo

    def wpack(w, ncols):
        # [2048, ncols] -> [128, pair, hi/lo, 2, ncols] fp8.
        # Weights are pre-scaled by 64 so the lo (residual) plane stays
        # clear of e4m3's subnormal floor; rmsnorm makes q/kv scale-
        # invariant, krope and the gate divide it back out on device.
        hi, lo = hilo(w * np.float32(64.0))
        s = np.stack([hi.reshape(8, 2, 128, ncols),
                      lo.reshape(8, 2, 128, ncols)], axis=1)
        return np.ascontiguousarray(s.transpose(3, 0, 1, 2, 4))

    # d-major x per batch: hi|lo fp8, xt[t, p, l*2048 + k*128+c]
    xts = []
    for b in range(B):
        a = xf[b].reshape(NT, 128, 16, 128).transpose(0, 3, 2, 1)
        hi, lo = hilo(a.reshape(NT, 128, D_MODEL))
        xts.append(np.ascontiguousarray(
            np.concatenate([hi, lo], axis=-1)))

    # pre-swizzle (T, d) -> (128, NT*d) partition-major, pack with brk
    cos_pm = cost.reshape(NT, 128, 32).transpose(1, 0, 2).reshape(128, 512)
    sin_pm = sint.reshape(NT, 128, 32).transpose(1, 0, 2).reshape(128, 512)
    masks_pk = np.ascontiguousarray(np.concatenate([mdiag, mfar], axis=1))

    in_maps = []
    for c in range(NCORES):
        b, g = c // N_KV, c % N_KV
        qcols = wqkv_f[:, 4 * g * 128:(4 * g + 4) * 128]
        kvcols = wqkv_f[:, (N_HEADS + g) * 128:(N_HEADS + g + 1) * 128]
        gcols = wg_f[:, 4 * g * 128:(4 * g + 4) * 128]
        wall = np.concatenate([qcols, kvcols, wrk_f, gcols[:, 0:64]], axis=1)
        crow = scale[:, None] * scaler[None, 4 * g:4 * g + 4]
        crow_pm = np.ascontiguousarray(
            crow.reshape(NT, 128, HPC).transpose(1, 0, 2)).reshape(128, 64)
        trig = np.concatenate(
            [cos_pm, sin_pm, crow_pm, brk_t], axis=1).astype(np.float32)
        in_maps.append({
            "xt": xts[b],
            "wqkv": wpack(wall, 768),
            "wg": wpack(np.ascontiguousarray(gcols[:, 64:512]), 448),
            "trig": np.ascontiguousarray(trig),
            "masks": masks_pk,
            "ident": ident,
        })
    return in_maps


def kernel(hidden_states, W_qkv, W_rk, b_rk, softmax_scaler, W_g):
    from concourse.bass_utils import run_bass_kernel_spmd

    nc = _build_nc()
    in_maps = _host_inputs(hidden_states, W_qkv, W_rk, b_rk,
                           softmax_scaler, W_g)
    res = run_bass_kernel_spmd(nc, in_maps, list(range(NCORES)))
    outf = np.empty((B, T, N_HEADS, D_HEAD), np.float32)
    for c in range(NCORES):
        b, g = c // N_KV, c % N_KV
        outf[b, :, 4 * g:4 * g + 4, :] = res.results[c]["out"].reshape(
            T, HPC, D_HEAD)
    return outf



# revision 91
# speedup vs baseline: 1.0203x; 1.0203x over previous
"""Trainium2 Bass kernel for nn_MixerGroupedTiedAttention.

Sharding: 8 cores = (batch B=2) x (kv-group G=4). Each core handles one
batch element and one group of 4 q-heads + their shared kv-head:
  - qkv / gate projections: tensor-parallel column slices of W_qkv / W_g
  - k_rope (head-tied) replicated (folded into the per-core W slab)
  - sliding-window attention (W=1024) computed block-sparse over 128x128
    token tiles.

Design notes:
  - x is transposed to d-major on the HOST and shipped per token-tile,
    so the PE transposes + evac copies of a device-side transpose
    pipeline vanish.
  - Projections run as fp8e4m3 DoubleRow matmuls (0.5 PE cycles/row,
    2x the 16-bit rate) over 256-deep chunk pairs. Precision comes
    from a hi+lo split of BOTH operands (lo = fp8 of the residual;
    3 cross terms accumulate in PSUM, the ~1e-3 lo*lo term is
    dropped). Weights are pre-scaled by 64 so the lo plane clears
    e4m3's subnormal floor; rmsnorm makes q/kv scale-invariant and
    krope/gate divide the 64 back out in fused scalar ops. Attention
    stays fp16/bf16 (q/k fp16, probs/V bf16). rel err ~8.7e-3 vs the
    2e-2 gate.
  - Single fused pass per 128-token tile t: projections (tokens on PSUM
    partitions) -> rmsnorm/rope/scales -> transposes -> attention ROW
    t-1 at lag 1: per 128x128 key block one score matmul computes ALL
    4 heads (the 4-head q strip is the moving operand), exp'd into a
    per-block [j, (h,i)] bf16 strip; y accumulates per head over the 9
    blocks with a ones column fused into v giving the softmax
    denominator for free. The rmsnorm/rope chain + transposes run
    under tc.high_priority() — they are the cross-block critical path
    (they free the pq PSUM bank and gate the next row's scores).
  - The last row borrows the four then-dead projection PSUM banks to
    accumulate y key-block-major, so after the final exp strip only 4
    matmuls remain before the normalize/store.
  - DMA: HWDGE costs a serialized ~625ns per DMA, so transfers are
    batched >=1.5KB/partition; cost-model bandwidth is ~360GB/s shared.
"""

import numpy as np
import ml_dtypes

D_MODEL = 2048
N_HEADS = 16
N_KV = 4
D_HEAD = 128
D1 = 64
D2 = 64
WSIZE = 1024
EPS = 1e-6
ROPE_BASE = 10000.0
B = 2
T = 2048
NCORES = 8
HPC = 4  # q heads per core
NT = T // 128  # 16 token tiles
NWB = WSIZE // 128 + 1  # 9 key tiles per query tile
WCOLS = 1280  # q(512) | kv(128) | krope(64) | pad(64) | gate(512)

_BF16 = ml_dtypes.bfloat16
_FP8 = ml_dtypes.float8_e4m3  # matches mybir dt.float8e4
_built = {}


def _build_nc():
    """Build the single-core SPMD Bass program (same program all 8 cores)."""
    if "nc" in _built:
        return _built["nc"]
    import concourse.bacc as bacc
    import concourse.tile as tile
    from concourse import mybir

    # All ACT functions this kernel uses (Copy/Square/Ln/Exp) live in the
    # "natural_log_exp_and_others" table set. The table-load pass greedily
    # picks the first set containing each function, which alternates table
    # loads (~2.7us each) between sets; restrict every other set's
    # advertised membership so exactly one table set is ever loaded.
    if not getattr(bacc, "_act_tables_pinned", False):
        _orig_gat = bacc.get_activation_tables
        _mine = {
            mybir.ActivationFunctionType.Copy,
            mybir.ActivationFunctionType.Identity,
            mybir.ActivationFunctionType.Square,
            mybir.ActivationFunctionType.Ln,
            mybir.ActivationFunctionType.Exp,
        }

        def _pinned_gat(arch):
            tabs = _orig_gat(arch)
            return {
                name: (funcs if name == "natural_log_exp_and_others"
                       else funcs - _mine)
                for name, funcs in tabs.items()
            }

        bacc.get_activation_tables = _pinned_gat
        bacc._act_tables_pinned = True

    f32 = mybir.dt.float32
    f32r = mybir.dt.float32r
    f16 = mybir.dt.float16
    bf16 = mybir.dt.bfloat16
    AF = mybir.ActivationFunctionType
    OP = mybir.AluOpType

    nc = bacc.Bacc("TRN2", target_bir_lowering=False, debug=False)

    def din(name, shape, dt):
        return nc.dram_tensor(name, shape, dt, kind="ExternalInput").ap()

    fp8 = mybir.dt.float8e4
    DR = mybir.MatmulPerfMode.DoubleRow

    # x and W ship as fp8e4m3 hi+lo pairs (hi = quantized value, lo =
    # quantized residual); projections run as DoubleRow matmuls over
    # 256-deep chunk pairs with 3 cross terms (hh, hl, lh — the lo*lo
    # term is ~1e-3 relative and dropped), 1.33x faster than 16-bit.
    # x layout per tile: [128, hi(2048) | lo(2048)] d-major; weights
    # pair-major [128, pair, hi/lo, 2, cols] so a DoubleRow rhs slice
    # is contiguous. trig packs cos | sin | crow | brk; masks mdiag|mfar.
    xt = din("xt", [NT, 128, 2 * D_MODEL], fp8)
    wqkv = din("wqkv", [128, 8, 2, 2, 768], fp8)  # q|kv|krope|g0:64
    wg = din("wg", [128, 8, 2, 2, 448], fp8)      # gate cols 64:512
    trig = din("trig", [128, 1152], f32)
    masks = din("masks", [128, 1024], bf16)    # tiled x4 for fused heads
    ident = din("ident", [128, 128], f16)
    out = nc.dram_tensor("out", [T, 512], f32, kind="ExternalOutput").ap()

    with tile.TileContext(nc) as tc:
        with tc.tile_pool(name="persist", bufs=1) as pp:
            wq_sb = pp.tile([128, 8, 2, 2, 768], fp8, name="wq")
            wg_sb = pp.tile([128, 8, 2, 2, 448], fp8, name="wg")
            trig_sb = pp.tile([128, 1152], f32)
            mask_sb = pp.tile([128, 1024], bf16)
            ident_sb = pp.tile([128, 128], f16)
            eps_sb = pp.tile([128, 1], f32)
            kT_sb = pp.tile([128, T], f16)
            vaug_sb = pp.tile([128, NT, 132], bf16)
            nc.vector.memset(eps_sb[:], EPS)
            # ones column of v_aug (softmax-denominator accumulator)
            nc.vector.memset(vaug_sb[:, :, 128], 1.0)

            def cosv(t):
                return trig_sb[:, t * 32:(t + 1) * 32]

            def sinv(t):
                return trig_sb[:, 512 + t * 32:512 + (t + 1) * 32]

            def crowv(t):
                return trig_sb[:, 1024 + t * 4:1024 + (t + 1) * 4]

            brk_sb = trig_sb[:, 1088:1152]
            mdiag_sb = mask_sb[:, 0:512]
            mfar_sb = mask_sb[:, 512:1024]

            with tc.tile_pool(name="xt", bufs=8) as xtp, \
                 tc.tile_pool(name="qt", bufs=3) as qtp, \
                 tc.tile_pool(name="gs", bufs=4) as gsp, \
                 tc.tile_pool(name="u", bufs=19) as up, \
                 tc.tile_pool(name="aw", bufs=6) as awp, \
                 tc.tile_pool(name="qk", bufs=3) as qkp, \
                 tc.tile_pool(name="bw", bufs=3) as bwp, \
                 tc.tile_pool(name="stg", bufs=2) as stp, \
                 tc.tile_pool(name="ppq", bufs=2, space="PSUM") as ppq, \
                 tc.tile_pool(name="ppkv", bufs=1, space="PSUM") as ppkv, \
                 tc.tile_pool(name="ppg", bufs=1, space="PSUM") as ppg, \
                 tc.tile_pool(name="psS", bufs=4, space="PSUM") as psS:

                # ---- DMA schedule. HWDGE costs a serialized ~625ns per
                # DMA regardless of size, so batch: weight chunks stream in
                # pairs/quads, x tiles 0-2 in 1024-col halves, small
                # tensors packed (trig, masks). Cold start interleaves the
                # qkv weight stream with x tiles 0-2 just in time for the
                # trailing cold matmuls. ----
                xpre = {}
                TERMS = ((0, 0), (0, 1), (1, 0))  # (x hi/lo, w hi/lo)

                def fetch_x(t, pieces=1):
                    xs = xtp.tile([128, 2 * D_MODEL], fp8, name=f"x_{t}",
                                  tag="x")
                    xpre[t] = xs
                    for c in range(pieces):
                        w = 2 * D_MODEL // pieces
                        nc.sync.dma_start(xs[:, c * w:(c + 1) * w],
                                          xt[t][:, c * w:(c + 1) * w])

                for t in range(3):
                    fetch_x(t, pieces=0)  # allocate in consumption order
                xcold = [xpre[t] for t in range(3)]

                def xpiece(t, c):
                    # piece 0 = hi plane, piece 1 = lo plane (2KB each)
                    nc.sync.dma_start(xcold[t][:, c * 2048:(c + 1) * 2048],
                                      xt[t][:, c * 2048:(c + 1) * 2048])

                def xdr(x_slab, xi, P):
                    """DoubleRow stationary: chunk pair P of x hi/lo."""
                    base = xi * D_MODEL + P * 256
                    return x_slab[:, base:base + 256].rearrange(
                        "p (i c) -> p i c", i=2)

                def wqs(p0, p1):
                    nc.sync.dma_start(wq_sb[:, p0:p1], wqkv[:, p0:p1])

                wqs(0, 1)
                xpiece(0, 0)
                wqs(1, 2)
                xpiece(0, 1)
                wqs(2, 3)
                xpiece(1, 0)
                wqs(3, 4)
                xpiece(1, 1)
                wqs(4, 5)
                xpiece(2, 0)
                wqs(5, 6)
                xpiece(2, 1)
                wqs(6, 7)
                wqs(7, 8)
                nc.sync.dma_start(wg_sb[:, 0:2], wg[:, 0:2])
                nc.sync.dma_start(ident_sb[:], ident[:])
                nc.sync.dma_start(trig_sb[:], trig[:])
                nc.sync.dma_start(wg_sb[:, 2:4], wg[:, 2:4])
                fetch_x(3)
                nc.sync.dma_start(mask_sb[:], masks[:])
                nc.sync.dma_start(wg_sb[:, 4:6], wg[:, 4:6])
                fetch_x(4)
                nc.sync.dma_start(wg_sb[:, 6:8], wg[:, 6:8])
                fetch_x(5)
                fetch_x(6)
                for t in range(7, NT):
                    fetch_x(t)

                qf_t = {}
                kpre_t = {}
                gs_t = {}

                def dr_group(dst, x_slab, w_sb, c0, c1):
                    """One 256-deep-pair DoubleRow accumulation group."""
                    n = 0
                    for P in range(8):
                        for xi, wi in TERMS:
                            nc.tensor.matmul(
                                dst, xdr(x_slab, xi, P),
                                w_sb[:, P, wi, :, c0:c1],
                                start=(n == 0), stop=(n == 23),
                                perf_mode=DR)
                            n += 1

                def emit_proj_qkv(t):
                    """q/kv projection matmuls for tile t (PE only)."""
                    x_slab = xpre[t]
                    pq = ppq.tile([128, 512], f32, name=f"pq_{t}", tag="pq")
                    pkv = ppkv.tile([128, 256], f32, name=f"pk_{t}",
                                    tag="pk")
                    dr_group(pq[:, 0:256], x_slab, wq_sb, 0, 256)
                    dr_group(pq[:, 256:512], x_slab, wq_sb, 256, 512)
                    dr_group(pkv[:], x_slab, wq_sb, 512, 768)
                    return pq, pkv

                def emit_proj_gate(t):
                    """gate projection matmuls for tile t (PE only).
                    Gate cols 0:64 ride in the kvp region (former pad);
                    this computes cols 64:512."""
                    x_slab = xpre.pop(t)
                    pg = ppg.tile([128, 448], f32, name=f"pg_{t}", tag="pg")
                    dr_group(pg[:, 0:256], x_slab, wg_sb, 0, 256)
                    dr_group(pg[:, 256:448], x_slab, wg_sb, 256, 448)
                    return pg

                def emit_chain_qkv(t, pq, pkv):
                    """rmsnorm/rope/scales for tile t (no PE ops).

                    DVE order puts the rope muls (which depend only on the
                    just-landed PSUM + trig) before anything that needs the
                    ACT rmsnorm chain, so DVE and ACT start in parallel the
                    moment the projection completes."""
                    cos_t = cosv(t)
                    sin_t = sinv(t)
                    cosb = cos_t[:, None, :].broadcast_to([128, 4, 32])
                    sinb = sin_t[:, None, :].broadcast_to([128, 4, 32])
                    qh = pq[:].rearrange("p (h d) -> p h d", h=4)
                    t1 = awp.tile([128, 4, 32], f32, name=f"t1_{t}", tag="t1")
                    t2 = awp.tile([128, 4, 32], f32, name=f"t2_{t}", tag="t2")
                    t3 = awp.tile([128, 4, 32], f32, name=f"t3_{t}", tag="t3")
                    t4 = awp.tile([128, 4, 32], f32, name=f"t4_{t}", tag="t4")
                    rp = awp.tile([128, 4, 64], f32, name=f"rp_{t}", tag="rp")
                    nc.vector.tensor_mul(t1[:], qh[:, :, 64:96], cosb)
                    nc.vector.tensor_mul(t2[:], qh[:, :, 96:128], sinb)
                    nc.gpsimd.tensor_add(rp[:, :, 0:32], t1[:], t2[:])
                    nc.vector.tensor_mul(t3[:], qh[:, :, 96:128], cosb)
                    nc.vector.tensor_mul(t4[:], qh[:, :, 64:96], sinb)
                    nc.gpsimd.tensor_sub(rp[:, :, 32:64], t3[:], t4[:])
                    # k_rope: bias, rope (no norm)
                    krf = awp.tile([128, 64], f32, name=f"krf_{t}", tag="krf")
                    nc.vector.scalar_tensor_tensor(
                        krf[:], pkv[:, 128:192], 1.0 / 64.0, brk_sb,
                        OP.mult, OP.add)
                    k1 = awp.tile([128, 32], f32, name=f"k1_{t}", tag="k1")
                    k2 = awp.tile([128, 32], f32, name=f"k2_{t}", tag="k2")
                    k3 = awp.tile([128, 32], f32, name=f"k3_{t}", tag="k3")
                    k4 = awp.tile([128, 32], f32, name=f"k4_{t}", tag="k4")
                    kpre = qkp.tile([128, 128], f16, name=f"kp_{t}",
                                    tag="kp")
                    nc.vector.tensor_mul(k1[:], krf[:, 0:32], cos_t)
                    nc.vector.tensor_mul(k2[:], krf[:, 32:64], sin_t)
                    nc.gpsimd.tensor_add(kpre[:, 64:96], k1[:], k2[:])
                    nc.vector.tensor_mul(k3[:], krf[:, 32:64], cos_t)
                    nc.vector.tensor_mul(k4[:], krf[:, 0:32], sin_t)
                    nc.gpsimd.tensor_sub(kpre[:, 96:128], k3[:], k4[:])

                    # rmsnorm scales: sumsq over each 128-wide head chunk
                    # (chunks 0-3 = q heads, 4 = kv head); r = rsqrt(mean+
                    # eps) via exp(-0.5*ln(.)) — keeps every ACT function in
                    # the ln/exp table set
                    # one wide Square (no per-op accumulator-read tax)
                    # + DVE segmented reduce replaces 5 accum activations
                    ss = awp.tile([128, 5], f32, name=f"ss_{t}", tag="ss")
                    sq = awp.tile([128, 512], f32, name=f"sq_{t}", tag="sq")
                    sqk = awp.tile([128, 128], f32, name=f"sqk_{t}",
                                   tag="sqk")
                    nc.scalar.activation(sq[:], pq[:], AF.Square)
                    nc.scalar.activation(sqk[:], pkv[:, 0:128], AF.Square)
                    nc.vector.tensor_reduce(
                        ss[:, 0:4], sq[:].rearrange("p (h d) -> p h d", h=4),
                        mybir.AxisListType.X, OP.add)
                    nc.vector.tensor_reduce(
                        ss[:, 4:5], sqk[:], mybir.AxisListType.X, OP.add)
                    lnm = awp.tile([128, 5], f32, name=f"lnm_{t}", tag="lnm")
                    nc.scalar.activation(lnm[:], ss[:], AF.Ln,
                                         scale=1.0 / 128.0, bias=eps_sb[:])
                    r = awp.tile([128, 5], f32, name=f"r_{t}", tag="r")
                    nc.scalar.activation(r[:], lnm[:], AF.Exp, scale=-0.5)
                    rc = awp.tile([128, HPC], f32, name=f"rc_{t}", tag="rc")
                    nc.vector.tensor_mul(rc[:], r[:, 0:4], crowv(t))
                    qf = qkp.tile([128, 4, 128], f16, name=f"qf_{t}",
                                  tag="qf")
                    rcb = rc[:, :, None].broadcast_to([128, 4, 64])
                    nc.vector.tensor_mul(qf[:, :, 0:64], qh[:, :, 0:64], rcb)
                    nc.vector.tensor_mul(qf[:, :, 64:128], rp[:], rcb)
                    qf_t[t] = qf

                    # kv head -> v (token-major) and k tied half
                    nc.vector.tensor_scalar(
                        vaug_sb[:, t, 0:128], pkv[:, 0:128],
                        r[:, 4:5], None, OP.mult)
                    nc.vector.tensor_scalar(
                        kpre[:, 0:64], pkv[:, 0:64],
                        r[:, 4:5], None, OP.mult)
                    kpre_t[t] = kpre

                qT_t = {}

                def emit_trans(t):
                    """PE transposes of qf/kpre for tile t; evac on DVE
                    (qT) and Pool (kT slab) keeps ACT free for exps."""
                    qf = qf_t.pop(t)
                    kpre = kpre_t.pop(t)
                    tq = psS.tile([128, 640], f16, name=f"tq_{t}", tag="s")
                    for h in range(HPC):
                        nc.tensor.transpose(
                            tq[:, h * 128:(h + 1) * 128], qf[:, h, :],
                            ident_sb[:])
                    nc.tensor.transpose(tq[:, 512:640], kpre[:], ident_sb[:])
                    qT = qtp.tile([128, HPC, 128], f16, name=f"qT_{t}",
                                  tag="qT")
                    # kT first: the diag score block waits on it, while
                    # qT has the whole y(t-1) stretch of slack. DVE (not
                    # Pool): GPSIMD cannot read PSUM on real HW.
                    nc.vector.tensor_copy(kT_sb[:, t * 128:(t + 1) * 128],
                                          tq[:, 512:640])
                    nc.vector.tensor_copy(
                        qT[:], tq[:, 0:512].rearrange("p (h t) -> p h t",
                                                      h=4))
                    qT_t[t] = qT

                def emit_chain_gate(t, pkv, pg):
                    # gate: silu = g / (1 + exp(-g)) — exp keeps the single
                    # ACT table set; +1 on Pool; reciprocal+mul on DVE.
                    # g cols 0:64 come from the kvp PSUM (former pad cols).
                    gsg = awp.tile([128, 512], f32, name=f"gsg_{t}",
                                   tag="gsg")
                    nc.scalar.activation(gsg[:, 0:64], pkv[:, 192:256],
                                         AF.Exp, scale=-1.0 / 64.0)
                    nc.scalar.activation(gsg[:, 64:512], pg[:], AF.Exp,
                                         scale=-1.0 / 64.0)
                    gw = awp.tile([128, 512], f32, name=f"gw_{t}", tag="gw")
                    nc.gpsimd.tensor_scalar_add(gw[:], gsg[:], 1.0)
                    gwi = awp.tile([128, 512], f32, name=f"gwi_{t}",
                                   tag="gwi")
                    nc.vector.reciprocal(gwi[:], gw[:])
                    g16 = gsp.tile([128, 512], f16, name=f"gs_{t}", tag="gs")
                    nc.vector.scalar_tensor_tensor(
                        g16[:, 0:64], pkv[:, 192:256], 1.0 / 64.0,
                        gwi[:, 0:64], OP.mult, OP.mult)
                    nc.vector.scalar_tensor_tensor(
                        g16[:, 64:512], pg[:], 1.0 / 64.0,
                        gwi[:, 64:512], OP.mult, OP.mult)
                    gs_t[t] = g16

                ublks_t = {}

                def emit_b1s(r):
                    """Attention row r: scores + exp strips. Natural block
                    order — the diag block (needing the freshest kT slab)
                    goes last, giving the Pool kT evac time to land. The
                    mask multiplies are deferred to emit_b2 (on Pool) so
                    the DVE queue never stalls waiting for the last exp."""
                    qT = qT_t.pop(r)
                    j0 = max(0, r - (NWB - 1))
                    nblk = r - j0 + 1
                    # scores: per key block, ONE matmul for all 4 heads
                    # (moving = the 4-head q strip); exp to a [j,(h,i)] strip
                    ublks = [None] * nblk
                    for wi in range(nblk):
                        tj = j0 + wi
                        s_ps = psS.tile([128, 512], f32,
                                        name=f"s_{r}_{wi}", tag="s")
                        nc.tensor.matmul(
                            s_ps[:], kT_sb[:, tj * 128:(tj + 1) * 128],
                            qT[:], start=True, stop=True)
                        u_t = up.tile([128, 512], bf16, name=f"u_{r}_{wi}",
                                      tag="u")
                        nc.scalar.activation(u_t[:], s_ps[:], AF.Exp)
                        ublks[wi] = u_t
                    # masks trail all exps: far block first (its exp landed
                    # long ago), diag last
                    if nblk == NWB:
                        nc.vector.tensor_mul(ublks[0][:], ublks[0][:],
                                             mfar_sb)
                    nc.vector.tensor_mul(ublks[nblk - 1][:],
                                         ublks[nblk - 1][:], mdiag_sb)
                    ublks_t[r] = ublks

                def emit_b2(r, ypools=None):
                    """Attention row r: y accumulation, normalize, gate, out.

                    Default: head-major accumulation through the shared psS
                    rotation. For the last row, `ypools` borrows the four
                    then-dead projection banks so accumulation can run
                    key-block-major (4 concurrent groups, one per bank) —
                    after the final exp strip only 4 matmuls remain."""
                    j0 = max(0, r - (NWB - 1))
                    nblk = r - j0 + 1
                    ublks = ublks_t.pop(r)
                    gsr = gs_t.pop(r)
                    stage = stp.tile([128, 512], f32, name=f"o_{r}", tag="o")
                    if ypools is None:
                        yps = []
                        for h in range(HPC):
                            y_ps = psS.tile([128, 132], f32,
                                            name=f"y_{r}_{h}", tag="s")
                            yps.append(y_ps)
                            for wi in range(nblk):
                                nc.tensor.matmul(
                                    y_ps[:, 0:129],
                                    ublks[wi][:, h * 128:(h + 1) * 128],
                                    vaug_sb[:, j0 + wi, 0:129],
                                    start=(wi == 0), stop=(wi == nblk - 1))
                    else:
                        yps = [pool.tile([128, 132], f32, name=f"y_{r}_{h}",
                                         tag=tag)
                               for h, (pool, tag) in enumerate(ypools)]
                        for wi in range(nblk):
                            for h in range(HPC):
                                nc.tensor.matmul(
                                    yps[h][:, 0:129],
                                    ublks[wi][:, h * 128:(h + 1) * 128],
                                    vaug_sb[:, j0 + wi, 0:129],
                                    start=(wi == 0), stop=(wi == nblk - 1))
                    for h in range(HPC):
                        linv = bwp.tile([128, 1], f32, name=f"li_{r}_{h}",
                                        tag="li")
                        nc.vector.reciprocal(linv[:], yps[h][:, 128:129])
                        nc.vector.scalar_tensor_tensor(
                            stage[:, h * 128:(h + 1) * 128],
                            yps[h][:, 0:128], linv[:],
                            gsr[:, h * 128:(h + 1) * 128],
                            OP.mult, OP.mult)
                        if h == 1:
                            nc.sync.dma_start(
                                out[r * 128:(r + 1) * 128, 0:256],
                                stage[:, 0:256])
                    nc.sync.dma_start(out[r * 128:(r + 1) * 128, 256:512],
                                      stage[:, 256:512])

                # ---- cold start: tiles 0-2 q/kv chunk-major with
                # trailing offsets (PE tracks the weight stream); their
                # q/kv PSUM partly borrowed from the idle psS pool ----
                cold_q = [ppq.tile([128, 512], f32, name="cq_0", tag="pq"),
                          ppq.tile([128, 512], f32, name="cq_1", tag="pq"),
                          psS.tile([128, 512], f32, name="cq_2", tag="s")]
                cold_kv = [ppkv.tile([128, 256], f32, name="ck_0", tag="pk"),
                           psS.tile([128, 256], f32, name="ck_1", tag="s"),
                           psS.tile([128, 256], f32, name="ck_2", tag="s")]
                # interleave only the kv + q-lo-half groups while tracking
                # the weight stream (they accumulate in different PSUM
                # banks; the q halves share one bank so their groups must
                # be sequential), then burst the q-hi-half groups
                for i in range(12):
                    for t, trail in ((0, 0), (1, 2), (2, 4)):
                        P = i - trail
                        if not (0 <= P < 8):
                            continue
                        for dst, c0, c1 in (
                                (cold_kv[t][:], 512, 768),
                                (cold_q[t][:, 0:256], 0, 256)):
                            for xi, wi in TERMS:
                                nc.tensor.matmul(
                                    dst, xdr(xpre[t], xi, P),
                                    wq_sb[:, P, wi, :, c0:c1],
                                    start=(P == 0 and (xi, wi) == TERMS[0]),
                                    stop=(P == 7 and (xi, wi) == TERMS[-1]),
                                    perf_mode=DR)
                for t in range(3):
                    dr_group(cold_q[t][:, 256:512], xpre[t], wq_sb, 256, 512)
                # cold epilogue: chains + transposes for tiles 0-2 ordered
                # so the borrowed psS slots free in pool-rotation order,
                # then attention rows start. Steady state runs at lag 1 —
                # row t-1's scores/exps/y hide under tile t's projections —
                # so only row 15's attention remains after the last proj.
                emit_chain_qkv(0, cold_q[0], cold_kv[0])
                emit_trans(0)
                emit_chain_qkv(1, cold_q[1], cold_kv[1])
                pg0 = emit_proj_gate(0)
                emit_chain_gate(0, cold_kv[0][:], pg0)
                emit_chain_qkv(2, cold_q[2], cold_kv[2])
                emit_trans(1)
                pg1 = emit_proj_gate(1)
                emit_chain_gate(1, cold_kv[1][:], pg1)
                emit_trans(2)
                emit_b1s(0)
                pg2 = emit_proj_gate(2)
                emit_chain_gate(2, cold_kv[2][:], pg2)
                emit_b2(0)
                emit_b1s(1)
                emit_b2(1)
                for t in range(3, NT - 1):
                    emit_b1s(t - 1)
                    pq, pkv = emit_proj_qkv(t)
                    pg = emit_proj_gate(t)
                    emit_b2(t - 1)
                    # the rmsnorm/rope chain + transposes are the cross-
                    # block critical path (they free the pq bank and gate
                    # the next row's scores); high priority lets them
                    # preempt the exp strips in the ready heaps
                    with tc.high_priority():
                        emit_chain_qkv(t, pq, pkv)
                        emit_trans(t)
                    emit_chain_gate(t, pkv, pg)
                # t = 15: row-15 scores jump ahead of row 14's y and the
                # gate silu jumps ahead of the row-15 exps on ACT, so the
                # tail (row 15's exps + y + normalize) starts as early as
                # possible after the last projection
                emit_b1s(NT - 2)
                pq, pkv = emit_proj_qkv(NT - 1)
                pg = emit_proj_gate(NT - 1)
                with tc.high_priority():
                    emit_chain_qkv(NT - 1, pq, pkv)
                    emit_trans(NT - 1)
                emit_chain_gate(NT - 1, pkv, pg)
                emit_b1s(NT - 1)
                emit_b2(NT - 2)
                emit_b2(NT - 1, ypools=[(ppq, "pq"), (ppq, "pq"),
                                        (ppkv, "pk"), (ppg, "pg")])

    nc.compile()
    _built["nc"] = nc
    return nc


def _host_inputs(hidden_states, W_qkv, W_rk, b_rk, softmax_scaler, W_g):
    """Per-core input dicts (host-side sharding / layout / dtype prep)."""
    inv_freq = 1.0 / (ROPE_BASE ** (np.arange(0, D2, 2, dtype=np.float32) / D2))
    tpos = np.arange(T, dtype=np.float32)
    freqs = tpos[:, None] * inv_freq[None, :]
    cost = np.cos(freqs).astype(np.float32)
    sint = np.sin(freqs).astype(np.float32)
    logpos = np.log(np.minimum(tpos + 1.0, float(WSIZE))).astype(np.float32)
    scale = logpos / np.float32(np.sqrt(D_HEAD))

    ii = np.arange(128)
    mdiag = np.tile((ii[:, None] <= ii[None, :]).astype(_BF16), (1, 4))
    mfar = np.tile((ii[:, None] >= ii[None, :]).astype(_BF16), (1, 4))
    ident = np.eye(128, dtype=np.float16)
    brk_t = np.broadcast_to(
        np.asarray(b_rk, np.float32)[None, :], (128, 64)).copy()

    xf = np.asarray(hidden_states, np.float32)
    wqkv_f = np.asarray(W_qkv, np.float32)
    wrk_f = np.asarray(W_rk, np.float32)
    wg_f = np.asarray(W_g, np.float32)
    scaler = np.asarray(softmax_scaler, np.float32)
    zpad = np.zeros((D_MODEL, 64), np.float32)

    def hilo(a):
        hi = a.astype(_FP8)
        lo = (a - hi.astype(np.float32)).astype(_FP8)
        return hi, lo

    def wpack(w, ncols):
        # [2048, ncols] -> [128, pair, hi/lo, 2, ncols] fp8.
        # Weights are pre-scaled by 64 so the lo (residual) plane stays
        # clear of e4m3's subnormal floor; rmsnorm makes q/kv scale-
        # invariant, krope and the gate divide it back out on device.
        hi, lo = hilo(w * np.float32(64.0))
        s = np.stack([hi.reshape(8, 2, 128, ncols),
                      lo.reshape(8, 2, 128, ncols)], axis=1)
        return np.ascontiguousarray(s.transpose(3, 0, 1, 2, 4))

    # d-major x per batch: hi|lo fp8, xt[t, p, l*2048 + k*128+c]
    xts = []
    for b in range(B):
        a = xf[b].reshape(NT, 128, 16, 128).transpose(0, 3, 2, 1)
        hi, lo = hilo(a.reshape(NT, 128, D_MODEL))
        xts.append(np.ascontiguousarray(
            np.concatenate([hi, lo], axis=-1)))

    # pre-swizzle (T, d) -> (128, NT*d) partition-major, pack with brk
    cos_pm = cost.reshape(NT, 128, 32).transpose(1, 0, 2).reshape(128, 512)
    sin_pm = sint.reshape(NT, 128, 32).transpose(1, 0, 2).reshape(128, 512)
    masks_pk = np.ascontiguousarray(np.concatenate([mdiag, mfar], axis=1))

    in_maps = []
    for c in range(NCORES):
        b, g = c // N_KV, c % N_KV
        qcols = wqkv_f[:, 4 * g * 128:(4 * g + 4) * 128]
        kvcols = wqkv_f[:, (N_HEADS + g) * 128:(N_HEADS + g + 1) * 128]
        gcols = wg_f[:, 4 * g * 128:(4 * g + 4) * 128]
        wall = np.concatenate([qcols, kvcols, wrk_f, gcols[:, 0:64]], axis=1)
        crow = scale[:, None] * scaler[None, 4 * g:4 * g + 4]
        crow_pm = np.ascontiguousarray(
            crow.reshape(NT, 128, HPC).transpose(1, 0, 2)).reshape(128, 64)
        trig = np.concatenate(
            [cos_pm, sin_pm, crow_pm, brk_t], axis=1).astype(np.float32)
        in_maps.append({
            "xt": xts[b],
            "wqkv": wpack(wall, 768),
            "wg": wpack(np.ascontiguousarray(gcols[:, 64:512]), 448),
            "trig": np.ascontiguousarray(trig),
            "masks": masks_pk,
            "ident": ident,
        })
    return in_maps


def kernel(hidden_states, W_qkv, W_rk, b_rk, softmax_scaler, W_g):
    from concourse.bass_utils import run_bass_kernel_spmd

    nc = _build_nc()
    in_maps = _host_inputs(hidden_states, W_qkv, W_rk, b_rk,
                           softmax_scaler, W_g)
    res = run_bass_kernel_spmd(nc, in_maps, list(range(NCORES)))
    outf = np.empty((B, T, N_HEADS, D_HEAD), np.float32)
    for c in range(NCORES):
        b, g = c // N_KV, c % N_KV
        outf[b, :, 4 * g:4 * g + 4, :] = res.results[c]["out"].reshape(
            T, HPC, D_HEAD)
    return outf

